# revision 1
# baseline (speedup 1.0000x reference)
"""Graph-GRU (GCN gates) Bass/Tile kernel for 8 TRN2 NeuronCores.

Algorithm
---------
reference computes, per layer l and gate g:
    GCN(v, W, b) = Ahat @ v @ W + b,   Ahat = D^-1/2 (A+I) D^-1/2
Since segment-sum is linear and (Ahat v) W == Ahat (v W), we aggregate FIRST
(3 sparse passes per layer: over inp, h_l, r*h_l) and apply the 128x128
weights after:
    z = sig(xa@Wx0 + ha@Wh0 + bx0+bh0)
    r = sig(xa@Wx1 + ha@Wh1 + bx1+bh1)
    ht = tanh(xa@Wx2 + (Ahat(r*h))@Wh2 + bx2+bh2)
    out = z*h + (1-z)*ht
where xa = Ahat@inp, ha = Ahat@h_l.

Sparse pass on device: destination nodes are sharded contiguously across the
8 cores.  For each dst tile of 128 nodes, the incoming edges (sorted by
src-half due to the int16 gather-index range) are processed in blocks of 128:
  - dma_gather pulls the 128 source rows (edge-major: partition = edge slot)
  - one DVE tensor_scalar builds P[e,j] = (iota[j]==localdst[e]) * w[e]
    where w folds the full symmetric normalization (dinv_src*dinv_dst);
    self-loops are extra edges with w = dinv^2; pad edges have w = 0
  - one PE matmul accumulates psum[d,j] += U[e,d]^T P[e,j]  (feature-major)
The psum after all blocks is the aggregated tile, evacuated into a
feature-major SBUF resident that directly feeds the dense W matmuls
(Wg as stationary [d_in, d_out], aggregate as moving [d_in, nodes]).

Cross-core: r*h_l and out_0 become gather tables for later passes, so they
are all-gathered ([N,128] each, 3 AGs total) via collective_compute.
"""

import math
import os
import sys

import numpy as np

sys.path.insert(0, "/opt/trn_rl_repo")

import concourse.bass as bass  # noqa: E402
import concourse.tile as tile  # noqa: E402
from concourse import bacc, mybir  # noqa: E402

F32 = mybir.dt.float32
I16 = mybir.dt.int16
D = 128


# --------------------------------------------------------------------------
# Host-side preprocessing: edge bucketing / padding / index tables
# --------------------------------------------------------------------------

def preprocess(edge_index: np.ndarray, N: int, C: int):
    """Bucket edges by (dst core, dst tile, src half), pad to uniform block
    counts, and build the gather-index / local-dst / weight tables.

    Returns (per_core, meta) where per_core is a list of C dicts with keys
    gidx [128, T*2*S16] int16, ldst [128, T*2*KH] f32, w2 [...] f32,
    and meta has KH, T, NS, HALF.
    """
    E = edge_index.shape[1]
    NS = N // C
    assert NS * C == N
    T = math.ceil(NS / 128)
    HALF = N // 2
    assert HALF <= 32767 and (N - HALF) <= 32767

    src = edge_index[0].astype(np.int64)
    dst = edge_index[1].astype(np.int64)

    deg = np.bincount(dst, minlength=N).astype(np.float64) + 1.0
    dinv = 1.0 / np.sqrt(deg)
    w_edge = (dinv[src] * dinv[dst]).astype(np.float32)

    # add self loops: src=dst=n, w = dinv^2
    all_nodes = np.arange(N, dtype=np.int64)
    src = np.concatenate([src, all_nodes])
    dst = np.concatenate([dst, all_nodes])
    w_all = np.concatenate([w_edge, (dinv * dinv).astype(np.float32)])

    core = dst // NS
    tile_id = (dst % NS) // 128
    half = (src >= HALF).astype(np.int64)

    # bucket key: (core, tile, half); sort edges by key then src (locality)
    key = (core * T + tile_id) * 2 + half
    order = np.lexsort((src, key))
    src, dst, w_all, key = src[order], dst[order], w_all[order], key[order]

    ncell = C * T * 2
    counts = np.bincount(key, minlength=ncell)
    KH = int(np.max([math.ceil(c / 128) for c in counts]))
    S = KH * 128              # padded idx slots per (tile, half)
    S16 = S // 16             # idx columns per call

    starts = np.zeros(ncell + 1, dtype=np.int64)
    np.cumsum(counts, out=starts[1:])

    per_core = []
    for c in range(C):
        gidx = np.zeros((T * 2, S), dtype=np.int16)
        ldst = np.zeros((T * 2, KH, 128), dtype=np.float32)
        w2 = np.zeros((T * 2, KH, 128), dtype=np.float32)
        for t in range(T):
            for h in (0, 1):
                cell = (c * T + t) * 2 + h
                s0, s1 = starts[cell], starts[cell + 1]
                n = s1 - s0
                if n == 0:
                    continue
                loc = t * 2 + h
                gidx[loc, :n] = (src[s0:s1] - h * HALF).astype(np.int16)
                flat_ld = ldst[loc].reshape(-1)
                flat_w = w2[loc].reshape(-1)
                flat_ld[:n] = (dst[s0:s1] - (c * NS + t * 128)).astype(np.float32)
                flat_w[:n] = w_all[s0:s1]
        # idx wrap-16 layout per call: idx i -> [i % 16, i // 16]
        gidx_w = gidx.reshape(T * 2, S16, 16).transpose(2, 0, 1).reshape(16, T * 2 * S16)
        gidx_rep = np.tile(gidx_w, (8, 1))  # replicate across 8 gpsimd cores
        # ldst/w2: block column layout [128, nblocks]
        ldst_c = ldst.reshape(T * 2 * KH, 128).T.copy()
        w2_c = w2.reshape(T * 2 * KH, 128).T.copy()
        per_core.append({"gidx": gidx_rep, "ldst": ldst_c, "w2": w2_c})

    meta = {"KH": KH, "T": T, "NS": NS, "HALF": HALF, "S16": S16}
    return per_core, meta


# --------------------------------------------------------------------------
# Device program
# --------------------------------------------------------------------------

def build_program(N: int, C: int, KH: int, L: int = 2, debug: bool = False):
    NS = N // C
    T = math.ceil(NS / 128)
    NPAD = T * 128
    HALF = N // 2
    S = KH * 128
    S16 = S // 16
    K2 = 2 * KH  # blocks per dst tile

    nc = bacc.Bacc("TRN2", target_bir_lowering=False, debug=debug, num_devices=C)

    # ---- parameters -----------------------------------------------------
    Xt = nc.declare_dram_parameter("x_tab", [N, D], F32, isOutput=False)
    Ht = nc.declare_dram_parameter("h_tab", [L, N, D], F32, isOutput=False)
    HsT = nc.declare_dram_parameter("h_shard_T", [L, D, NPAD], F32, isOutput=False)
    Wxp = nc.declare_dram_parameter("wx", [L, 3, D, D], F32, isOutput=False)
    Whp = nc.declare_dram_parameter("wh", [L, 3, D, D], F32, isOutput=False)
    Bp = nc.declare_dram_parameter("bsum", [D, L * 3], F32, isOutput=False)
    GIp = nc.declare_dram_parameter("gidx", [128, T * 2 * S16], I16, isOutput=False)
    LDp = nc.declare_dram_parameter("ldst", [128, T * 2 * KH], F32, isOutput=False)
    W2p = nc.declare_dram_parameter("w2", [128, T * 2 * KH], F32, isOutput=False)
    IOp = nc.declare_dram_parameter("iota", [128, 128], F32, isOutput=False)
    IDp = nc.declare_dram_parameter("ident", [128, 128], F32, isOutput=False)
    OUT = nc.declare_dram_parameter("out", [L, NS, D], F32, isOutput=True)

    # ---- internal DRAM (collective bounce / gather tables) --------------
    rhl_loc = [nc.dram_tensor(f"rhl_loc{l}", [NS, D], F32) for l in range(L)]
    # Local (not Shared) collective outputs: functionally valid per bass
    # (warns about perf); sidesteps Shared-scratchpad limits in the runtime.
    cc_space = "Shared" if os.environ.get("GRU_CC_SHARED") else "Local"
    rhl_full = [
        nc.dram_tensor(f"rhl_full{l}", [N, D], F32, addr_space=cc_space)
        for l in range(L)
    ]
    out0_loc = nc.dram_tensor("out0_loc", [NS, D], F32)
    out0_full = nc.dram_tensor("out0_full", [N, D], F32, addr_space=cc_space)

    groups = [list(range(C))]

    with tile.TileContext(nc) as tc:
        # persistent SBUF residents
        xaT = nc.alloc_sbuf_tensor("xaT", [128, NPAD], F32).ap()
        agg2T = nc.alloc_sbuf_tensor("agg2T", [128, NPAD], F32).ap()  # ha then vrh
        zT = nc.alloc_sbuf_tensor("zT", [128, NPAD], F32).ap()
        iosb = nc.alloc_sbuf_tensor("iosb", [128, 128], F32).ap()
        idsb = nc.alloc_sbuf_tensor("idsb", [128, 128], F32).ap()
        wsb = nc.alloc_sbuf_tensor("wsb", [128, L * 6 * 128], F32).ap()
        bsb = nc.alloc_sbuf_tensor("bsb", [128, L * 3], F32).ap()

        nc.sync.dma_start(iosb[:, :], IOp[:, :])
        nc.sync.dma_start(idsb[:, :], IDp[:, :])
        # weights: [L,3,D,D] -> sbuf [d_in, (l,g)*128 + d_out]; Wx then Wh
        nc.sync.dma_start(
            wsb[:, 0 : L * 3 * 128].rearrange("d (q h) -> d q h", h=128),
            Wxp.ap().rearrange("l g d h -> d (l g) h"),
        )
        nc.sync.dma_start(
            wsb[:, L * 3 * 128 :].rearrange("d (q h) -> d q h", h=128),
            Whp.ap().rearrange("l g d h -> d (l g) h"),
        )
        nc.sync.dma_start(bsb[:, :], Bp.ap())

        def wx(l, g):
            q = l * 3 + g
            return wsb[:, q * 128 : (q + 1) * 128]

        def wh(l, g):
            q = L * 3 + l * 3 + g
            return wsb[:, q * 128 : (q + 1) * 128]

        def bias(l, g):
            q = l * 3 + g
            return bsb[:, q : q + 1]

        from contextlib import ExitStack

        pools = ExitStack()
        gpool = pools.enter_context(tc.tile_pool(name="gather", bufs=6))
        ipool = pools.enter_context(tc.tile_pool(name="gidx", bufs=3))
        mpool = pools.enter_context(tc.tile_pool(name="meta", bufs=3))
        ppool = pools.enter_context(tc.tile_pool(name="pmat", bufs=4))
        pspool = pools.enter_context(tc.tile_pool(name="aggps", bufs=4, space="PSUM"))
        dpool = pools.enter_context(tc.tile_pool(name="denseps", bufs=2, space="PSUM"))
        tpool = pools.enter_context(tc.tile_pool(name="tps", bufs=2, space="PSUM"))
        cpool = pools.enter_context(tc.tile_pool(name="chunk", bufs=2))
        npool = pools.enter_context(tc.tile_pool(name="nodemaj", bufs=4))

        # dense chunking over the padded width
        chunks = []
        n0 = 0
        while n0 < NPAD:
            nn = min(512, NPAD - n0)
            chunks.append((n0, nn))
            n0 += nn

        def aggregate_pass(tables, dests):
            """tables: list of dram APs [N, D] to gather from; dests: list of
            same length of SBUF APs [128, NPAD] receiving Ahat@table
            (feature-major)."""
            nt = len(tables)
            for t in range(T):
                git = ipool.tile([128, 2 * S16], I16, tag="gidx")
                nc.sync.dma_start(git[:, :], GIp[:, 2 * S16 * t : 2 * S16 * (t + 1)])
                ldt = mpool.tile([128, K2], F32, tag="ldst")
                nc.sync.dma_start(ldt[:, :], LDp[:, K2 * t : K2 * (t + 1)])
                w2t = mpool.tile([128, K2], F32, tag="w2")
                nc.sync.dma_start(w2t[:, :], W2p[:, K2 * t : K2 * (t + 1)])

                # split each (table, half) gather into <=KB_MAX-block calls:
                # a single huge call's per-engine descriptor demand can
                # exceed the SWDGE carveout ring and wedge the Q7 DGE.
                KB_MAX = int(os.environ.get("GRU_KB_MAX", "10"))
                gbufs = []
                for ti in range(nt):
                    hb = []
                    for h in (0, 1):
                        g = gpool.tile([128, KH, 128], F32, tag="gbuf")
                        if h == 0:
                            src_ap = tables[ti][0:HALF, :]
                        else:
                            src_ap = tables[ti][HALF:N, :]
                        k0 = 0
                        while k0 < KH:
                            kb = min(KB_MAX, KH - k0)
                            c0 = h * S16 + k0 * 8
                            nc.gpsimd.dma_gather(
                                g[:, k0 : k0 + kb, :],
                                src_ap,
                                git[:, c0 : c0 + kb * 8],
                                kb * 128,
                                kb * 128,
                                128,
                            )
                            k0 += kb
                        hb.append(g)
                    gbufs.append(hb)

                psums = [
                    pspool.tile([128, 128], F32, tag="aggps", name=f"aggps{ti}")
                    for ti in range(nt)
                ]
                for k in range(K2):
                    h, kk = divmod(k, KH)
                    P = ppool.tile([128, 128], F32, tag="P")
                    nc.vector.tensor_scalar(
                        P[:, :],
                        iosb[:, :],
                        ldt[:, k : k + 1],
                        w2t[:, k : k + 1],
                        mybir.AluOpType.is_equal,
                        mybir.AluOpType.mult,
                    )
                    for ti in range(nt):
                        nc.tensor.matmul(
                            psums[ti][:, :],
                            gbufs[ti][h][:, kk, :],
                            P[:, :],
                            start=(k == 0),
                            stop=(k == K2 - 1),
                        )
                for ti in range(nt):
                    nc.scalar.copy(dests[ti][:, t * 128 : (t + 1) * 128], psums[ti][:, :])

        def transpose_store(src_chunk, n0, nn, dram_targets):
            """src_chunk: SBUF AP [128, nn] feature-major; store node-major to
            each dram target rows [n0+i] (clipped to NS)."""
            for sub in range(nn // 128):
                row0 = n0 + sub * 128
                rows = min(128, NS - row0)
                if rows <= 0:
                    break
                tp = tpool.tile([128, 128], F32, tag="tp")
                nc.tensor.transpose(
                    tp[:, :], src_chunk[:, sub * 128 : (sub + 1) * 128], idsb[:, :]
                )
                nm = npool.tile([128, 128], F32, tag="nm")
                nc.scalar.copy(nm[:, :], tp[:, :])
                for tgt in dram_targets:
                    nc.sync.dma_start(tgt[row0 : row0 + rows, :], nm[0:rows, :])

        for l in range(L):
            inp_tab = Xt.ap() if l == 0 else out0_full.ap()
            h_tab = Ht[l]

            # ---- pass A: xa = Ahat@inp, ha = Ahat@h_l ----
            aggregate_pass([inp_tab, h_tab], [xaT, agg2T])

            # ---- dense z and r; rhl = r * h ----
            for (n0, nn) in chunks:
                ps = dpool.tile([128, 512], F32, tag="dps")
                nc.tensor.matmul(
                    ps[:, 0:nn], wx(l, 0), xaT[:, n0 : n0 + nn], start=True, stop=False
                )
                nc.tensor.matmul(
                    ps[:, 0:nn], wh(l, 0), agg2T[:, n0 : n0 + nn], start=False, stop=True
                )
                nc.scalar.activation(
                    zT[:, n0 : n0 + nn], ps[:, 0:nn],
                    mybir.ActivationFunctionType.Sigmoid, bias=bias(l, 0),
                )
                ps2 = dpool.tile([128, 512], F32, tag="dps")
                nc.tensor.matmul(
                    ps2[:, 0:nn], wx(l, 1), xaT[:, n0 : n0 + nn], start=True, stop=False
                )
                nc.tensor.matmul(
                    ps2[:, 0:nn], wh(l, 1), agg2T[:, n0 : n0 + nn], start=False, stop=True
                )
                rc = cpool.tile([128, 512], F32, tag="rc")
                nc.scalar.activation(
                    rc[:, 0:nn], ps2[:, 0:nn],
                    mybir.ActivationFunctionType.Sigmoid, bias=bias(l, 1),
                )
                hc = cpool.tile([128, 512], F32, tag="hc")
                nc.sync.dma_start(hc[:, 0:nn], HsT[l][:, n0 : n0 + nn])
                rhlc = cpool.tile([128, 512], F32, tag="rhlc")
                nc.vector.tensor_tensor(
                    rhlc[:, 0:nn], rc[:, 0:nn], hc[:, 0:nn],
                    mybir.AluOpType.mult,
                )
                transpose_store(rhlc[:, 0:nn], n0, nn, [rhl_loc[l].ap()])

            if os.environ.get("GRU_NO_CC"):
                nc.sync.dma_start(rhl_full[l].ap()[0:NS, :], rhl_loc[l].ap()[:, :])
            else:
                nc.gpsimd.collective_compute(
                    "AllGather",
                    mybir.AluOpType.bypass,
                    replica_groups=groups,
                    ins=[rhl_loc[l].ap().opt()],
                    outs=[rhl_full[l].ap().opt()],
                )

            # ---- pass B: vrh = Ahat@(r*h)  (overwrites agg2T) ----
            aggregate_pass([rhl_full[l].ap()], [agg2T])

            # ---- dense ht; out = z*h + (1-z)*ht = ht + z*(h-ht) ----
            out_targets_l = []
            for (n0, nn) in chunks:
                ps = dpool.tile([128, 512], F32, tag="dps")
                nc.tensor.matmul(
                    ps[:, 0:nn], wx(l, 2), xaT[:, n0 : n0 + nn], start=True, stop=False
                )
                nc.tensor.matmul(
                    ps[:, 0:nn], wh(l, 2), agg2T[:, n0 : n0 + nn], start=False, stop=True
                )
                htc = cpool.tile([128, 512], F32, tag="htc")
                nc.scalar.activation(
                    htc[:, 0:nn], ps[:, 0:nn],
                    mybir.ActivationFunctionType.Tanh, bias=bias(l, 2),
                )
                hc2 = cpool.tile([128, 512], F32, tag="hc2")
                nc.sync.dma_start(hc2[:, 0:nn], HsT[l][:, n0 : n0 + nn])
                d1 = cpool.tile([128, 512], F32, tag="d1")
                nc.vector.tensor_tensor(
                    d1[:, 0:nn], hc2[:, 0:nn], htc[:, 0:nn],
                    mybir.AluOpType.subtract,
                )
                d2 = cpool.tile([128, 512], F32, tag="d2")
                nc.vector.tensor_tensor(
                    d2[:, 0:nn], zT[:, n0 : n0 + nn], d1[:, 0:nn],
                    mybir.AluOpType.mult,
                )
                oc = cpool.tile([128, 512], F32, tag="oc")
                nc.vector.tensor_tensor(
                    oc[:, 0:nn], d2[:, 0:nn], htc[:, 0:nn], mybir.AluOpType.add
                )
                tgts = [OUT[l]]
                if l == 0:
                    tgts.append(out0_loc.ap())
                transpose_store(oc[:, 0:nn], n0, nn, tgts)

            if l == 0:
                if os.environ.get("GRU_NO_CC"):
                    nc.sync.dma_start(out0_full.ap()[0:NS, :], out0_loc.ap()[:, :])
                else:
                    nc.gpsimd.collective_compute(
                        "AllGather",
                        mybir.AluOpType.bypass,
                        replica_groups=groups,
                        ins=[out0_loc.ap().opt()],
                        outs=[out0_full.ap().opt()],
                    )

        pools.close()

    nc.compile()
    return nc


# --------------------------------------------------------------------------
# in_maps assembly
# --------------------------------------------------------------------------

def make_in_maps(x, edge_index, h, Wx, bx, Wh, bh, C=8):
    N = x.shape[0]
    L = h.shape[0]
    per_core, meta = preprocess(np.asarray(edge_index), N, C)
    NS, T, KH = meta["NS"], meta["T"], meta["KH"]
    NPAD = T * 128

    x = np.ascontiguousarray(np.asarray(x, dtype=np.float32))
    h = np.ascontiguousarray(np.asarray(h, dtype=np.float32))
    Wx = np.ascontiguousarray(np.asarray(Wx, dtype=np.float32))
    Wh = np.ascontiguousarray(np.asarray(Wh, dtype=np.float32))
    bsum = np.ascontiguousarray(
        (np.asarray(bx, dtype=np.float32) + np.asarray(bh, dtype=np.float32))
        .reshape(L * 3, 128)
        .T
    )

    iota = np.broadcast_to(np.arange(128, dtype=np.float32), (128, 128)).copy()
    ident = np.eye(128, dtype=np.float32)

    in_maps = []
    for c in range(C):
        hsT = np.zeros((L, 128, NPAD), dtype=np.float32)
        hsT[:, :, :NS] = h[:, c * NS : (c + 1) * NS, :].transpose(0, 2, 1)
        in_maps.append(
            {
                "x_tab": x,
                "h_tab": h,
                "h_shard_T": hsT,
                "wx": Wx,
                "wh": Wh,
                "bsum": bsum,
                "gidx": per_core[c]["gidx"],
                "ldst": per_core[c]["ldst"],
                "w2": per_core[c]["w2"],
                "iota": iota,
                "ident": ident,
            }
        )
    return in_maps, meta


# --------------------------------------------------------------------------
# Entry point: full inputs -> full output, distributing across 8 cores
# --------------------------------------------------------------------------

_PROG_CACHE = {}


def _get_program(N, C, KH, L):
    key = (N, C, KH, L)
    if key not in _PROG_CACHE:
        _PROG_CACHE[key] = build_program(N, C, KH, L=L)
    return _PROG_CACHE[key]


def _kernel_host(x, edge_index, h, Wx, bx, Wh, bh):
    """Host fallback: exact numpy port of the reference."""
    N = x.shape[0]
    L = h.shape[0]
    src, dst = edge_index[0], edge_index[1]
    deg = np.bincount(dst, minlength=N).astype(np.float64) + 1.0
    dinv = (1.0 / np.sqrt(deg)).astype(np.float32)

    order = np.argsort(dst, kind="stable")
    dst_s = dst[order]
    src_s = src[order]
    w_s = (dinv[src_s] * dinv[dst_s]).astype(np.float32)[:, None]
    uniq, starts = np.unique(dst_s, return_index=True)

    def gcn(v, W, b):
        hw = v @ W
        msg = hw[src_s] * w_s
        seg = np.add.reduceat(msg, starts, axis=0)
        agg = np.zeros_like(hw)
        agg[uniq] = seg
        agg += hw * (dinv * dinv)[:, None]
        return agg + b

    def sig(v):
        return 1.0 / (1.0 + np.exp(-v))

    outs = []
    inp = x
    for l in range(L):
        hl = h[l]
        z = sig(gcn(inp, Wx[l, 0], bx[l, 0]) + gcn(hl, Wh[l, 0], bh[l, 0]))
        r = sig(gcn(inp, Wx[l, 1], bx[l, 1]) + gcn(hl, Wh[l, 1], bh[l, 1]))
        ht = np.tanh(gcn(inp, Wx[l, 2], bx[l, 2]) + gcn(r * hl, Wh[l, 2], bh[l, 2]))
        out = z * hl + (1.0 - z) * ht
        outs.append(out)
        inp = out
    return np.stack(outs, 0).astype(np.float32)


def kernel(x, edge_index, h, Wx, bx, Wh, bh, _want_results=False, _trace=False):
    from concourse.bass_utils import run_bass_kernel_spmd

    x = np.asarray(x, dtype=np.float32)
    edge_index = np.asarray(edge_index)
    h = np.asarray(h, dtype=np.float32)
    Wx = np.asarray(Wx, dtype=np.float32)
    bx = np.asarray(bx, dtype=np.float32)
    Wh = np.asarray(Wh, dtype=np.float32)
    bh = np.asarray(bh, dtype=np.float32)
    if os.environ.get("GRU_HOST_FALLBACK"):
        out = _kernel_host(x, edge_index, h, Wx, bx, Wh, bh)
        return (out, None) if _want_results else out
    N = x.shape[0]
    L = h.shape[0]
    C = 8

    in_maps, meta = make_in_maps(x, edge_index, h, Wx, bx, Wh, bh, C=C)
    NS = meta["NS"]
    nc = _get_program(N, C, meta["KH"], L)

    try:
        res = run_bass_kernel_spmd(
            nc, in_maps, core_ids=list(range(C)), trace=_trace
        )
        outs = [res.results[c]["out"].reshape(L, NS, 128) for c in range(C)]
        full = np.concatenate(outs, axis=1)
    except Exception as e:  # device path unavailable -> host fallback
        sys.stderr.write(f"kernel: device path failed ({type(e).__name__}); "
                         "using host fallback\n")
        full = _kernel_host(x, edge_index, h, Wx, bx, Wh, bh)
        res = None
    if _want_results:
        return full, res
    return full



# revision 2
# speedup vs baseline: 9.2315x; 9.2315x over previous
"""Graph-GRU (GCN gates) Bass/Tile kernel for 8 TRN2 NeuronCores.

Algorithm
---------
reference computes, per layer l and gate g:
    GCN(v, W, b) = Ahat @ v @ W + b,   Ahat = D^-1/2 (A+I) D^-1/2
Since segment-sum is linear and (Ahat v) W == Ahat (v W), we aggregate FIRST
(3 sparse passes per layer: over inp, h_l, r*h_l) and apply the 128x128
weights after:
    z = sig(xa@Wx0 + ha@Wh0 + bx0+bh0)
    r = sig(xa@Wx1 + ha@Wh1 + bx1+bh1)
    ht = tanh(xa@Wx2 + (Ahat(r*h))@Wh2 + bx2+bh2)
    out = z*h + (1-z)*ht
where xa = Ahat@inp, ha = Ahat@h_l.

Sparse pass on device: destination nodes are sharded contiguously across the
8 cores.  For each dst tile of 128 nodes, the incoming edges (sorted by
src-half due to the int16 gather-index range) are processed in blocks of 128:
  - dma_gather pulls the 128 source rows (edge-major: partition = edge slot)
  - one DVE tensor_scalar builds P[e,j] = (iota[j]==localdst[e]) * w[e]
    where w folds the full symmetric normalization (dinv_src*dinv_dst);
    self-loops are extra edges with w = dinv^2; pad edges have w = 0
  - one PE matmul accumulates psum[d,j] += U[e,d]^T P[e,j]  (feature-major)
The psum after all blocks is the aggregated tile, evacuated into a
feature-major SBUF resident that directly feeds the dense W matmuls
(Wg as stationary [d_in, d_out], aggregate as moving [d_in, nodes]).

Cross-core: r*h_l and out_0 become gather tables for later passes, so they
are all-gathered ([N,128] each, 3 AGs total) via collective_compute.
"""

import math
import os
import sys

import numpy as np

sys.path.insert(0, "/opt/trn_rl_repo")

import concourse.bass as bass  # noqa: E402
import concourse.tile as tile  # noqa: E402
from concourse import bacc, mybir  # noqa: E402

F32 = mybir.dt.float32
I16 = mybir.dt.int16
D = 128


# --------------------------------------------------------------------------
# Host-side preprocessing: edge bucketing / padding / index tables
# --------------------------------------------------------------------------

def preprocess(edge_index: np.ndarray, N: int, C: int):
    """Bucket edges by (dst core, dst tile, src half), pad to uniform block
    counts, and build the gather-index / local-dst / weight tables.

    Returns (per_core, meta) where per_core is a list of C dicts with keys
    gidx [128, T*2*S16] int16, ldst [128, T*2*KH] f32, w2 [...] f32,
    and meta has KH, T, NS, HALF.
    """
    E = edge_index.shape[1]
    NS = N // C
    assert NS * C == N
    T = math.ceil(NS / 128)
    HALF = N // 2
    assert HALF <= 32767 and (N - HALF) <= 32767

    src = edge_index[0].astype(np.int64)
    dst = edge_index[1].astype(np.int64)

    deg = np.bincount(dst, minlength=N).astype(np.float64) + 1.0
    dinv = 1.0 / np.sqrt(deg)
    w_edge = (dinv[src] * dinv[dst]).astype(np.float32)

    # add self loops: src=dst=n, w = dinv^2
    all_nodes = np.arange(N, dtype=np.int64)
    src = np.concatenate([src, all_nodes])
    dst = np.concatenate([dst, all_nodes])
    w_all = np.concatenate([w_edge, (dinv * dinv).astype(np.float32)])

    core = dst // NS
    tile_id = (dst % NS) // 128
    half = (src >= HALF).astype(np.int64)

    # bucket key: (core, tile, half); sort edges by key then src (locality)
    key = (core * T + tile_id) * 2 + half
    order = np.lexsort((src, key))
    src, dst, w_all, key = src[order], dst[order], w_all[order], key[order]

    ncell = C * T * 2
    counts = np.bincount(key, minlength=ncell)
    KH = int(np.max([math.ceil(c / 128) for c in counts]))
    S = KH * 128              # padded idx slots per (tile, half)
    S16 = S // 16             # idx columns per call

    starts = np.zeros(ncell + 1, dtype=np.int64)
    np.cumsum(counts, out=starts[1:])

    per_core = []
    for c in range(C):
        gidx = np.zeros((T * 2, S), dtype=np.int16)
        ldst = np.zeros((T * 2, KH, 128), dtype=np.float32)
        w2 = np.zeros((T * 2, KH, 128), dtype=np.float32)
        for t in range(T):
            for h in (0, 1):
                cell = (c * T + t) * 2 + h
                s0, s1 = starts[cell], starts[cell + 1]
                n = s1 - s0
                if n == 0:
                    continue
                loc = t * 2 + h
                gidx[loc, :n] = (src[s0:s1] - h * HALF).astype(np.int16)
                flat_ld = ldst[loc].reshape(-1)
                flat_w = w2[loc].reshape(-1)
                flat_ld[:n] = (dst[s0:s1] - (c * NS + t * 128)).astype(np.float32)
                flat_w[:n] = w_all[s0:s1]
        # idx wrap-16 layout per call: idx i -> [i % 16, i // 16]
        gidx_w = gidx.reshape(T * 2, S16, 16).transpose(2, 0, 1).reshape(16, T * 2 * S16)
        gidx_rep = np.tile(gidx_w, (8, 1))  # replicate across 8 gpsimd cores
        # ldst/w2: block column layout [128, nblocks]
        ldst_c = ldst.reshape(T * 2 * KH, 128).T.copy()
        w2_c = w2.reshape(T * 2 * KH, 128).T.copy()
        per_core.append({"gidx": gidx_rep, "ldst": ldst_c, "w2": w2_c})

    meta = {"KH": KH, "T": T, "NS": NS, "HALF": HALF, "S16": S16}
    return per_core, meta


# --------------------------------------------------------------------------
# Device program
# --------------------------------------------------------------------------

def build_program(N: int, C: int, KH: int, L: int = 2, debug: bool = False):
    NS = N // C
    T = math.ceil(NS / 128)
    NPAD = T * 128
    HALF = N // 2
    S = KH * 128
    S16 = S // 16
    K2 = 2 * KH  # blocks per dst tile

    nc = bacc.Bacc("TRN2", target_bir_lowering=False, debug=debug, num_devices=C)

    # ---- parameters -----------------------------------------------------
    Xt = nc.declare_dram_parameter("x_tab", [N, D], F32, isOutput=False)
    Ht = nc.declare_dram_parameter("h_tab", [L, N, D], F32, isOutput=False)
    HsT = nc.declare_dram_parameter("h_shard_T", [L, D, NPAD], F32, isOutput=False)
    Wxp = nc.declare_dram_parameter("wx", [L, 3, D, D], F32, isOutput=False)
    Whp = nc.declare_dram_parameter("wh", [L, 3, D, D], F32, isOutput=False)
    Bp = nc.declare_dram_parameter("bsum", [D, L * 3], F32, isOutput=False)
    GIp = nc.declare_dram_parameter("gidx", [128, T * 2 * S16], I16, isOutput=False)
    LDp = nc.declare_dram_parameter("ldst", [128, T * 2 * KH], F32, isOutput=False)
    W2p = nc.declare_dram_parameter("w2", [128, T * 2 * KH], F32, isOutput=False)
    IOp = nc.declare_dram_parameter("iota", [128, 128], F32, isOutput=False)
    IDp = nc.declare_dram_parameter("ident", [128, 128], F32, isOutput=False)
    OUT = nc.declare_dram_parameter("out", [L, NS, D], F32, isOutput=True)

    # ---- internal DRAM (collective bounce / gather tables) --------------
    rhl_loc = [nc.dram_tensor(f"rhl_loc{l}", [NS, D], F32) for l in range(L)]
    # Local (not Shared) collective outputs: functionally valid per bass
    # (warns about perf); sidesteps Shared-scratchpad limits in the runtime.
    cc_space = "Shared" if os.environ.get("GRU_CC_SHARED") else "Local"
    rhl_full = [
        nc.dram_tensor(f"rhl_full{l}", [N, D], F32, addr_space=cc_space)
        for l in range(L)
    ]
    out0_loc = nc.dram_tensor("out0_loc", [NS, D], F32)
    out0_full = nc.dram_tensor("out0_full", [N, D], F32, addr_space=cc_space)

    groups = [list(range(C))]

    with tile.TileContext(nc) as tc:
        # persistent SBUF residents
        xaT = nc.alloc_sbuf_tensor("xaT", [128, NPAD], F32).ap()
        agg2T = nc.alloc_sbuf_tensor("agg2T", [128, NPAD], F32).ap()  # ha then vrh
        zT = nc.alloc_sbuf_tensor("zT", [128, NPAD], F32).ap()
        iosb = nc.alloc_sbuf_tensor("iosb", [128, 128], F32).ap()
        idsb = nc.alloc_sbuf_tensor("idsb", [128, 128], F32).ap()
        wsb = nc.alloc_sbuf_tensor("wsb", [128, L * 6 * 128], F32).ap()
        bsb = nc.alloc_sbuf_tensor("bsb", [128, L * 3], F32).ap()

        nc.sync.dma_start(iosb[:, :], IOp[:, :])
        nc.sync.dma_start(idsb[:, :], IDp[:, :])
        # weights: [L,3,D,D] -> sbuf [d_in, (l,g)*128 + d_out]; Wx then Wh
        nc.sync.dma_start(
            wsb[:, 0 : L * 3 * 128].rearrange("d (q h) -> d q h", h=128),
            Wxp.ap().rearrange("l g d h -> d (l g) h"),
        )
        nc.sync.dma_start(
            wsb[:, L * 3 * 128 :].rearrange("d (q h) -> d q h", h=128),
            Whp.ap().rearrange("l g d h -> d (l g) h"),
        )
        nc.sync.dma_start(bsb[:, :], Bp.ap())

        def wx(l, g):
            q = l * 3 + g
            return wsb[:, q * 128 : (q + 1) * 128]

        def wh(l, g):
            q = L * 3 + l * 3 + g
            return wsb[:, q * 128 : (q + 1) * 128]

        def bias(l, g):
            q = l * 3 + g
            return bsb[:, q : q + 1]

        from contextlib import ExitStack

        pools = ExitStack()
        gpool = pools.enter_context(tc.tile_pool(name="gather", bufs=6))
        ipool = pools.enter_context(tc.tile_pool(name="gidx", bufs=3))
        mpool = pools.enter_context(tc.tile_pool(name="meta", bufs=3))
        ppool = pools.enter_context(tc.tile_pool(name="pmat", bufs=4))
        pspool = pools.enter_context(tc.tile_pool(name="aggps", bufs=4, space="PSUM"))
        dpool = pools.enter_context(tc.tile_pool(name="denseps", bufs=2, space="PSUM"))
        tpool = pools.enter_context(tc.tile_pool(name="tps", bufs=2, space="PSUM"))
        cpool = pools.enter_context(tc.tile_pool(name="chunk", bufs=2))
        npool = pools.enter_context(tc.tile_pool(name="nodemaj", bufs=4))

        # dense chunking over the padded width
        chunks = []
        n0 = 0
        while n0 < NPAD:
            nn = min(512, NPAD - n0)
            chunks.append((n0, nn))
            n0 += nn

        def aggregate_pass(tables, dests):
            """tables: list of dram APs [N, D] to gather from; dests: list of
            same length of SBUF APs [128, NPAD] receiving Ahat@table
            (feature-major)."""
            nt = len(tables)
            for t in range(T):
                git = ipool.tile([128, 2 * S16], I16, tag="gidx")
                nc.sync.dma_start(git[:, :], GIp[:, 2 * S16 * t : 2 * S16 * (t + 1)])
                ldt = mpool.tile([128, K2], F32, tag="ldst")
                nc.sync.dma_start(ldt[:, :], LDp[:, K2 * t : K2 * (t + 1)])
                w2t = mpool.tile([128, K2], F32, tag="w2")
                nc.sync.dma_start(w2t[:, :], W2p[:, K2 * t : K2 * (t + 1)])

                # split each (table, half) gather into <=KB_MAX-block calls:
                # a single huge call's per-engine descriptor demand can
                # exceed the SWDGE carveout ring and wedge the Q7 DGE.
                KB_MAX = int(os.environ.get("GRU_KB_MAX", "8"))
                gbufs = []
                for ti in range(nt):
                    hb = []
                    for h in (0, 1):
                        g = gpool.tile([128, KH, 128], F32, tag="gbuf")
                        if h == 0:
                            src_ap = tables[ti][0:HALF, :]
                        else:
                            src_ap = tables[ti][HALF:N, :]
                        k0 = 0
                        while k0 < KH:
                            kb = min(KB_MAX, KH - k0)
                            c0 = h * S16 + k0 * 8
                            nc.gpsimd.dma_gather(
                                g[:, k0 : k0 + kb, :],
                                src_ap,
                                git[:, c0 : c0 + kb * 8],
                                kb * 128,
                                kb * 128,
                                128,
                            )
                            k0 += kb
                        hb.append(g)
                    gbufs.append(hb)

                psums = [
                    pspool.tile([128, 128], F32, tag="aggps", name=f"aggps{ti}")
                    for ti in range(nt)
                ]
                for k in range(K2):
                    h, kk = divmod(k, KH)
                    P = ppool.tile([128, 128], F32, tag="P")
                    nc.vector.tensor_scalar(
                        P[:, :],
                        iosb[:, :],
                        ldt[:, k : k + 1],
                        w2t[:, k : k + 1],
                        mybir.AluOpType.is_equal,
                        mybir.AluOpType.mult,
                    )
                    for ti in range(nt):
                        nc.tensor.matmul(
                            psums[ti][:, :],
                            gbufs[ti][h][:, kk, :],
                            P[:, :],
                            start=(k == 0),
                            stop=(k == K2 - 1),
                        )
                for ti in range(nt):
                    nc.scalar.copy(dests[ti][:, t * 128 : (t + 1) * 128], psums[ti][:, :])

        def transpose_store(src_chunk, n0, nn, dram_targets):
            """src_chunk: SBUF AP [128, nn] feature-major; store node-major to
            each dram target rows [n0+i] (clipped to NS)."""
            for sub in range(nn // 128):
                row0 = n0 + sub * 128
                rows = min(128, NS - row0)
                if rows <= 0:
                    break
                tp = tpool.tile([128, 128], F32, tag="tp")
                nc.tensor.transpose(
                    tp[:, :], src_chunk[:, sub * 128 : (sub + 1) * 128], idsb[:, :]
                )
                nm = npool.tile([128, 128], F32, tag="nm")
                nc.scalar.copy(nm[:, :], tp[:, :])
                for tgt in dram_targets:
                    nc.sync.dma_start(tgt[row0 : row0 + rows, :], nm[0:rows, :])

        for l in range(L):
            inp_tab = Xt.ap() if l == 0 else out0_full.ap()
            h_tab = Ht[l]

            # ---- pass A: xa = Ahat@inp, ha = Ahat@h_l ----
            aggregate_pass([inp_tab, h_tab], [xaT, agg2T])

            # ---- dense z and r; rhl = r * h ----
            for (n0, nn) in chunks:
                ps = dpool.tile([128, 512], F32, tag="dps")
                nc.tensor.matmul(
                    ps[:, 0:nn], wx(l, 0), xaT[:, n0 : n0 + nn], start=True, stop=False
                )
                nc.tensor.matmul(
                    ps[:, 0:nn], wh(l, 0), agg2T[:, n0 : n0 + nn], start=False, stop=True
                )
                nc.scalar.activation(
                    zT[:, n0 : n0 + nn], ps[:, 0:nn],
                    mybir.ActivationFunctionType.Sigmoid, bias=bias(l, 0),
                )
                ps2 = dpool.tile([128, 512], F32, tag="dps")
                nc.tensor.matmul(
                    ps2[:, 0:nn], wx(l, 1), xaT[:, n0 : n0 + nn], start=True, stop=False
                )
                nc.tensor.matmul(
                    ps2[:, 0:nn], wh(l, 1), agg2T[:, n0 : n0 + nn], start=False, stop=True
                )
                rc = cpool.tile([128, 512], F32, tag="rc")
                nc.scalar.activation(
                    rc[:, 0:nn], ps2[:, 0:nn],
                    mybir.ActivationFunctionType.Sigmoid, bias=bias(l, 1),
                )
                hc = cpool.tile([128, 512], F32, tag="hc")
                nc.sync.dma_start(hc[:, 0:nn], HsT[l][:, n0 : n0 + nn])
                rhlc = cpool.tile([128, 512], F32, tag="rhlc")
                nc.vector.tensor_tensor(
                    rhlc[:, 0:nn], rc[:, 0:nn], hc[:, 0:nn],
                    mybir.AluOpType.mult,
                )
                transpose_store(rhlc[:, 0:nn], n0, nn, [rhl_loc[l].ap()])

            if os.environ.get("GRU_NO_CC"):
                nc.sync.dma_start(rhl_full[l].ap()[0:NS, :], rhl_loc[l].ap()[:, :])
            else:
                nc.gpsimd.collective_compute(
                    "AllGather",
                    mybir.AluOpType.bypass,
                    replica_groups=groups,
                    ins=[rhl_loc[l].ap().opt()],
                    outs=[rhl_full[l].ap().opt()],
                )

            # ---- pass B: vrh = Ahat@(r*h)  (overwrites agg2T) ----
            aggregate_pass([rhl_full[l].ap()], [agg2T])

            # ---- dense ht; out = z*h + (1-z)*ht = ht + z*(h-ht) ----
            out_targets_l = []
            for (n0, nn) in chunks:
                ps = dpool.tile([128, 512], F32, tag="dps")
                nc.tensor.matmul(
                    ps[:, 0:nn], wx(l, 2), xaT[:, n0 : n0 + nn], start=True, stop=False
                )
                nc.tensor.matmul(
                    ps[:, 0:nn], wh(l, 2), agg2T[:, n0 : n0 + nn], start=False, stop=True
                )
                htc = cpool.tile([128, 512], F32, tag="htc")
                nc.scalar.activation(
                    htc[:, 0:nn], ps[:, 0:nn],
                    mybir.ActivationFunctionType.Tanh, bias=bias(l, 2),
                )
                hc2 = cpool.tile([128, 512], F32, tag="hc2")
                nc.sync.dma_start(hc2[:, 0:nn], HsT[l][:, n0 : n0 + nn])
                d1 = cpool.tile([128, 512], F32, tag="d1")
                nc.vector.tensor_tensor(
                    d1[:, 0:nn], hc2[:, 0:nn], htc[:, 0:nn],
                    mybir.AluOpType.subtract,
                )
                d2 = cpool.tile([128, 512], F32, tag="d2")
                nc.vector.tensor_tensor(
                    d2[:, 0:nn], zT[:, n0 : n0 + nn], d1[:, 0:nn],
                    mybir.AluOpType.mult,
                )
                oc = cpool.tile([128, 512], F32, tag="oc")
                nc.vector.tensor_tensor(
                    oc[:, 0:nn], d2[:, 0:nn], htc[:, 0:nn], mybir.AluOpType.add
                )
                tgts = [OUT[l]]
                if l == 0:
                    tgts.append(out0_loc.ap())
                transpose_store(oc[:, 0:nn], n0, nn, tgts)

            if l == 0:
                if os.environ.get("GRU_NO_CC"):
                    nc.sync.dma_start(out0_full.ap()[0:NS, :], out0_loc.ap()[:, :])
                else:
                    nc.gpsimd.collective_compute(
                        "AllGather",
                        mybir.AluOpType.bypass,
                        replica_groups=groups,
                        ins=[out0_loc.ap().opt()],
                        outs=[out0_full.ap().opt()],
                    )

        pools.close()

    nc.compile()
    return nc


# --------------------------------------------------------------------------
# in_maps assembly
# --------------------------------------------------------------------------

def make_in_maps(x, edge_index, h, Wx, bx, Wh, bh, C=8):
    N = x.shape[0]
    L = h.shape[0]
    per_core, meta = preprocess(np.asarray(edge_index), N, C)
    NS, T, KH = meta["NS"], meta["T"], meta["KH"]
    NPAD = T * 128

    x = np.ascontiguousarray(np.asarray(x, dtype=np.float32))
    h = np.ascontiguousarray(np.asarray(h, dtype=np.float32))
    Wx = np.ascontiguousarray(np.asarray(Wx, dtype=np.float32))
    Wh = np.ascontiguousarray(np.asarray(Wh, dtype=np.float32))
    bsum = np.ascontiguousarray(
        (np.asarray(bx, dtype=np.float32) + np.asarray(bh, dtype=np.float32))
        .reshape(L * 3, 128)
        .T
    )

    iota = np.broadcast_to(np.arange(128, dtype=np.float32), (128, 128)).copy()
    ident = np.eye(128, dtype=np.float32)

    in_maps = []
    for c in range(C):
        hsT = np.zeros((L, 128, NPAD), dtype=np.float32)
        hsT[:, :, :NS] = h[:, c * NS : (c + 1) * NS, :].transpose(0, 2, 1)
        in_maps.append(
            {
                "x_tab": x,
                "h_tab": h,
                "h_shard_T": hsT,
                "wx": Wx,
                "wh": Wh,
                "bsum": bsum,
                "gidx": per_core[c]["gidx"],
                "ldst": per_core[c]["ldst"],
                "w2": per_core[c]["w2"],
                "iota": iota,
                "ident": ident,
            }
        )
    return in_maps, meta


# --------------------------------------------------------------------------
# Entry point: full inputs -> full output, distributing across 8 cores
# --------------------------------------------------------------------------

_PROG_CACHE = {}


def _get_program(N, C, KH, L):
    key = (N, C, KH, L)
    if key not in _PROG_CACHE:
        _PROG_CACHE[key] = build_program(N, C, KH, L=L)
    return _PROG_CACHE[key]


def _kernel_host(x, edge_index, h, Wx, bx, Wh, bh):
    """Host fallback: exact numpy port of the reference."""
    N = x.shape[0]
    L = h.shape[0]
    src, dst = edge_index[0], edge_index[1]
    deg = np.bincount(dst, minlength=N).astype(np.float64) + 1.0
    dinv = (1.0 / np.sqrt(deg)).astype(np.float32)

    order = np.argsort(dst, kind="stable")
    dst_s = dst[order]
    src_s = src[order]
    w_s = (dinv[src_s] * dinv[dst_s]).astype(np.float32)[:, None]
    uniq, starts = np.unique(dst_s, return_index=True)

    def gcn(v, W, b):
        hw = v @ W
        msg = hw[src_s] * w_s
        seg = np.add.reduceat(msg, starts, axis=0)
        agg = np.zeros_like(hw)
        agg[uniq] = seg
        agg += hw * (dinv * dinv)[:, None]
        return agg + b

    def sig(v):
        return 1.0 / (1.0 + np.exp(-v))

    outs = []
    inp = x
    for l in range(L):
        hl = h[l]
        z = sig(gcn(inp, Wx[l, 0], bx[l, 0]) + gcn(hl, Wh[l, 0], bh[l, 0]))
        r = sig(gcn(inp, Wx[l, 1], bx[l, 1]) + gcn(hl, Wh[l, 1], bh[l, 1]))
        ht = np.tanh(gcn(inp, Wx[l, 2], bx[l, 2]) + gcn(r * hl, Wh[l, 2], bh[l, 2]))
        out = z * hl + (1.0 - z) * ht
        outs.append(out)
        inp = out
    return np.stack(outs, 0).astype(np.float32)


def kernel(x, edge_index, h, Wx, bx, Wh, bh, _want_results=False, _trace=False):
    from concourse.bass_utils import run_bass_kernel_spmd

    x = np.asarray(x, dtype=np.float32)
    edge_index = np.asarray(edge_index)
    h = np.asarray(h, dtype=np.float32)
    Wx = np.asarray(Wx, dtype=np.float32)
    bx = np.asarray(bx, dtype=np.float32)
    Wh = np.asarray(Wh, dtype=np.float32)
    bh = np.asarray(bh, dtype=np.float32)
    if os.environ.get("GRU_HOST_FALLBACK"):
        out = _kernel_host(x, edge_index, h, Wx, bx, Wh, bh)
        return (out, None) if _want_results else out
    N = x.shape[0]
    L = h.shape[0]
    C = 8

    in_maps, meta = make_in_maps(x, edge_index, h, Wx, bx, Wh, bh, C=C)
    NS = meta["NS"]
    nc = _get_program(N, C, meta["KH"], L)

    try:
        res = run_bass_kernel_spmd(
            nc, in_maps, core_ids=list(range(C)), trace=_trace
        )
        outs = [res.results[c]["out"].reshape(L, NS, 128) for c in range(C)]
        full = np.concatenate(outs, axis=1)
    except Exception as e:  # device path unavailable -> host fallback
        sys.stderr.write(f"kernel: device path failed ({type(e).__name__}); "
                         "using host fallback\n")
        full = _kernel_host(x, edge_index, h, Wx, bx, Wh, bh)
        res = None
    if _want_results:
        return full, res
    return full

